# revision 1
# baseline (speedup 1.0000x reference)
"""LoRA-linear Trainium2 Bass kernel (bf16, software-pipelined, HAM warm-up).

Computes, for T adapters: out[t] = x @ W.T + (x @ A_t.T) @ B_t.T + bias
Output: [T, B, S, Dout] float32.

Sharding: data-parallel over tokens across 8 NeuronCores (2048 tokens/core);
W/bias/selected-LoRA replicated. Matmul inputs are cast to bf16 on the host
(halves load traffic, enables fast-weight-load); accumulation stays fp32.

Per-core layout puts Dout on PSUM partitions (out.T tiles [dout=128, tok]):
  lowT[32t+j, tok] = sum_d A_t[j,d] x[tok,d]   (PE, k-major so compute starts
              on the first arriving x k-tile)
  base.T[m] = W[m-tile] @ x.T  (PE, accumulate over 8 k-tiles; bias folded
              into the ScalarE PSUM->SBUF evacuation as a per-partition bias)
  delta.T[t,m] = B_t.T[:, m-tile].T-contract lowT_t  (K=16 row-group matmuls)
  out.T[t,m] = base.T[m] + delta.T[t,m]  (VectorE tensor_tensor, PSUM+SBUF)
Delta matmuls for dout-tile m-1 are emitted after the base matmuls of tile m
(software pipelining) so the PE never stalls waiting for VectorE adds and the
HAM clock gate stays warm. Stores are contiguous 1MB blocks of out.T; the
host transposes back.
"""

import sys

if "/opt/trn_rl_repo" not in sys.path:
    sys.path.insert(0, "/opt/trn_rl_repo")

from contextlib import ExitStack

import ml_dtypes
import numpy as np

import concourse.bacc as bacc
import concourse.bass as bass
import concourse.mybir as mybir
import concourse.tile as tile
from concourse import bass_utils

# Problem constants (hardcoded per spec).
B, S, DIN, DOUT, R, NL, T = 4, 4096, 1024, 1024, 16, 8, 4
NCORES = 8
NTOK = B * S                 # 16384
CTOK = NTOK // NCORES        # 2048 tokens per core
KT = DIN // 128              # 8 k-tiles
MT = DOUT // 128             # 8 dout-tiles
NC_CHUNK = CTOK // 512       # 4 token-chunks of 512

F32 = mybir.dt.float32
BF16 = mybir.dt.bfloat16
NPBF16 = ml_dtypes.bfloat16


def _build_program():
    nc = bacc.Bacc("TRN2", target_bir_lowering=False, debug=False,
                   num_devices=NCORES)

    xt = nc.dram_tensor("xt", [DIN, CTOK], BF16, kind="ExternalInput").ap()
    wt = nc.dram_tensor("wt", [DIN, DOUT], BF16, kind="ExternalInput").ap()
    atp = nc.dram_tensor("atp", [DIN, 128], BF16, kind="ExternalInput").ap()
    btp = nc.dram_tensor("btp", [128, DOUT], BF16, kind="ExternalInput").ap()
    biasc = nc.dram_tensor("biasc", [128, MT], F32, kind="ExternalInput").ap()
    out = nc.dram_tensor("out", [T, MT, 128, CTOK], F32,
                         kind="ExternalOutput").ap()

    with tile.TileContext(nc) as tc, ExitStack() as ctx:
        const = ctx.enter_context(tc.tile_pool(name="const", bufs=1))
        base_sb = ctx.enter_context(tc.tile_pool(name="base_sb", bufs=3))
        out_sb = ctx.enter_context(tc.tile_pool(name="out_sb", bufs=2))
        bp_ps = ctx.enter_context(tc.tile_pool(name="bp_ps", bufs=2, space="PSUM"))
        dp_ps = ctx.enter_context(tc.tile_pool(name="dp_ps", bufs=6, space="PSUM"))

        # Loads: small tiles go on the scalar HWDGE ring (own FIFO, lands in
        # ~1us) so warm-up/phase-1 start immediately; the two big tensors are
        # single strided DMAs on the sync ring (one completion receipt each
        # instead of 16 serialized ones).
        at_all = const.tile([128, KT * 128], BF16, tag="at")
        nc.scalar.dma_start(at_all.rearrange("p (k r) -> p k r", k=KT),
                            atp.rearrange("(k p) r -> p k r", p=128))
        bt_s = const.tile([128, DOUT], BF16, tag="bt")
        nc.scalar.dma_start(bt_s[:], btp[:, :])
        bias_s = const.tile([128, MT], F32, tag="bias")
        nc.scalar.dma_start(bias_s[:], biasc[:, :])
        at_t = [at_all[:, bass.ts(k, 128)] for k in range(KT)]
        xt_t = []
        wt_t = []
        for k in range(KT):
            tx = const.tile([128, CTOK], BF16, tag=f"xt{k}")
            nc.sync.dma_start(tx[:], xt[bass.ts(k, 128), :])
            xt_t.append(tx)
            tw = const.tile([128, DOUT], BF16, tag=f"wt{k}")
            nc.sync.dma_start(tw[:], wt[bass.ts(k, 128), :])
            wt_t.append(tw)
        lowT_s = const.tile([128, CTOK], BF16, tag="lowT")

        # Warm-up matmuls during the DMA prologue: the HAM clock gate needs
        # ~3.4us of sustained PE activity to unthrottle 1.2 -> 2.4 GHz, and
        # the load-paced opening would otherwise run the whole first ~30us of
        # real matmuls at half clock. ~170 N=128 matmuls on already-resident
        # tiles span the ~16us prologue.
        warm = dp_ps.tile([128, 128], F32, tag="dp", name="warm")
        for _ in range(140):
            nc.tensor.matmul(warm[:], at_t[0][:], bt_s[:, 0:128],
                             start=True, stop=True)

        # Phase 1 (k-major): lowT[32t+j, tok] = sum_d A_sel[t,j,d] x[tok,d].
        lps = [dp_ps.tile([128, 512], F32, tag="dp", name=f"lp{c}")
               for c in range(NC_CHUNK)]
        for k in range(KT):
            for c in range(NC_CHUNK):
                nc.tensor.matmul(
                    lps[c][:],
                    at_t[k][:],
                    xt_t[k][:, bass.ts(c, 512)],
                    start=(k == 0), stop=(k == KT - 1),
                )
        for c in range(NC_CHUNK):
            nc.scalar.copy(lowT_s[:, bass.ts(c, 512)], lps[c][:])

        # Phase 2, software-pipelined chunk-wise: base(m) chunk c is emitted
        # before delta(m-1) chunk c so the PE always has a dense base group to
        # chew while VectorE drains the previous delta bank.
        def emit_base_chunk(m, bsb, c):
            bp = bp_ps.tile([128, 512], F32, tag="bp", name=f"bp{m}_{c}")
            for k in range(KT):
                nc.tensor.matmul(
                    bp[:],
                    wt_t[k][:, bass.ts(m, 128)],
                    xt_t[k][:, bass.ts(c, 512)],
                    start=(k == 0), stop=(k == KT - 1),
                )
            # Evacuate with the per-partition bias folded in.
            nc.scalar.activation(
                bsb[:, bass.ts(c, 512)], bp[:],
                mybir.ActivationFunctionType.Identity,
                bias=bias_s[:, m:m + 1],
            )

        def emit_delta_chunk(m, bsb, ods, c):
            for t in range(T):
                dp = dp_ps.tile([128, 512], F32, tag="dp", name=f"dp{m}_{c}_{t}")
                nc.tensor.matmul(
                    dp[:],
                    bt_s[32 * t:32 * t + R, bass.ts(m, 128)],
                    lowT_s[32 * t:32 * t + R, bass.ts(c, 512)],
                    start=True, stop=True,
                    tile_position=(32 * t, 0),
                )
                nc.vector.tensor_add(
                    ods[t][:, bass.ts(c, 512)],
                    bsb[:, bass.ts(c, 512)], dp[:],
                )

        def make_ods(m):
            return [out_sb.tile([128, CTOK], F32, tag=f"od{t}", name=f"od{t}_{m}")
                    for t in range(T)]

        def store_ods(m, ods):
            for t in range(T):
                nc.sync.dma_start(out[t, m, :, :], ods[t][:])

        prev_bsb = None
        prev_ods = None
        for m in range(MT):
            bsb = base_sb.tile([128, CTOK], F32, tag="bsb", name=f"bsb{m}")
            for c in range(NC_CHUNK):
                emit_base_chunk(m, bsb, c)
                if prev_bsb is not None:
                    emit_delta_chunk(m - 1, prev_bsb, prev_ods, c)
            if prev_ods is not None:
                store_ods(m - 1, prev_ods)
            prev_bsb, prev_ods = bsb, make_ods(m)
        for c in range(NC_CHUNK):
            emit_delta_chunk(MT - 1, prev_bsb, prev_ods, c)
        store_ods(MT - 1, prev_ods)

    nc.compile()
    return nc


_NC = None


def _get_program():
    global _NC
    if _NC is None:
        _NC = _build_program()
    return _NC


def kernel(**inputs):
    x = np.ascontiguousarray(np.asarray(inputs["x"], dtype=np.float32))
    W = np.asarray(inputs["W"], dtype=np.float32)
    bias_v = np.asarray(inputs["bias"], dtype=np.float32)
    lora_A = np.asarray(inputs["lora_A"], dtype=np.float32)
    lora_B = np.asarray(inputs["lora_B"], dtype=np.float32)
    tuner_index = np.asarray(inputs["tuner_index"]).astype(np.int64)

    assert x.shape == (B, S, DIN) and W.shape == (DOUT, DIN)
    assert tuner_index.shape == (T,)

    A_sel = lora_A[tuner_index]          # [T, R, Din]
    B_sel = lora_B[tuner_index]          # [T, Dout, R]

    xT = np.ascontiguousarray(x.reshape(NTOK, DIN).T).astype(NPBF16)
    wt = np.ascontiguousarray(W.T).astype(NPBF16)       # [Din, Dout]
    atp = np.zeros((DIN, 128), NPBF16)
    atp.reshape(DIN, T, 32)[:, :, :R] = A_sel.transpose(2, 0, 1).astype(NPBF16)
    btp = np.zeros((128, DOUT), NPBF16)
    btp.reshape(T, 32, DOUT)[:, :R, :] = B_sel.transpose(0, 2, 1).astype(NPBF16)
    biasc = np.ascontiguousarray(bias_v.reshape(MT, 128).T)   # [128, MT]

    in_maps = []
    for c in range(NCORES):
        in_maps.append({
            "xt": np.ascontiguousarray(xT[:, c * CTOK:(c + 1) * CTOK]),
            "wt": wt,
            "atp": atp,
            "btp": btp,
            "biasc": biasc,
        })

    nc = _get_program()
    res = bass_utils.run_bass_kernel_spmd(nc, in_maps, core_ids=list(range(NCORES)))

    big = np.empty((T, MT, 128, NTOK), np.float32)
    for c in range(NCORES):
        big[:, :, :, c * CTOK:(c + 1) * CTOK] = res.results[c]["out"]
    # [T, m, p, tok] -> [T, tok, m*128+p]
    full = np.ascontiguousarray(big.transpose(0, 3, 1, 2))
    return full.reshape(T, B, S, DOUT)



# revision 6
# speedup vs baseline: 1.2286x; 1.2286x over previous
"""LoRA-linear Trainium2 Bass kernel (bf16 I/O, k-streamed prologue).

Computes, for T adapters: out[t] = x @ W.T + (x @ A_t.T) @ B_t.T + bias
Output: [T, B, S, Dout] float32 (device stores bf16; host upcasts).

Sharding: data-parallel over tokens across 8 NeuronCores (2048 tokens/core);
W/bias/selected-LoRA replicated. Matmul inputs are bf16 (host-cast);
accumulation stays fp32; outputs stored bf16 (abs error ~half-ulp(4.5)
≈ 0.008 ≪ the 0.09 budget at rel<2e-2).

Per-core layout puts Dout on PSUM partitions (out.T tiles [dout=128, tok]):
  lowT[32t+j, tok] = sum_d A_t[j,d] x[tok,d]   (PE, k-streamed)
  base.T[m]  = W[m-tile] @ x.T                 (PE, 8 k-tile accumulation)
  delta.T[t,m] = B_t.T row-group matmuls (K=16, tile_position=(32t,0), the
               four adapters issue back-to-back into distinct PSUM banks so
               3 run concurrently per the XBUS budget)
  out.T[t,m] = base.T[m] + delta.T[t,m]        (VectorE tensor_add)

Schedule:
  - Small tensors (A) load first on the sync ring so warm-up matmuls are not
    queued behind the 6 MB of x/W traffic; B/bias ride the scalar ring.
  - Phase A streams k-tiles: as (x_k, w_k) land, the low-rank matmuls and the
    first two base chunks accumulate k-outer, so the DMA prologue is filled
    with real PE work instead of pure warm-up.
  - Main loop per (m, c-chunk): 8 base matmuls -> 2 ScalarE activations
    evacuate base (bias folded) into a duplicated [128,1024] tile -> 4 delta
    matmuls -> 2 VectorE adds of FD=1024 (batching the four FD=512 adds into
    two halves the per-op PSUM overhead and keeps DVE off the critical path).
  - Stores are [128, 2048] bf16 blocks per (t, m), issued as each m finishes.
"""

import sys

if "/opt/trn_rl_repo" not in sys.path:
    sys.path.insert(0, "/opt/trn_rl_repo")

from contextlib import ExitStack

import ml_dtypes
import numpy as np

import concourse.bacc as bacc
import concourse.bass as bass
import concourse.mybir as mybir
import concourse.tile as tile
from concourse import bass_utils

# Problem constants (hardcoded per spec).
B, S, DIN, DOUT, R, NL, T = 4, 4096, 1024, 1024, 16, 8, 4
NCORES = 8
NTOK = B * S                 # 16384
CTOK = NTOK // NCORES        # 2048 tokens per core
KT = DIN // 128              # 8 k-tiles
MT = DOUT // 128             # 8 dout-tiles
NCH = CTOK // 512            # 4 token-chunks of 512

F32 = mybir.dt.float32
BF16 = mybir.dt.bfloat16
NPBF16 = ml_dtypes.bfloat16


def _build_program():
    nc = bacc.Bacc("TRN2", target_bir_lowering=False, debug=False,
                   num_devices=NCORES)

    xt = nc.dram_tensor("xt", [DIN, CTOK], BF16, kind="ExternalInput").ap()
    wt = nc.dram_tensor("wt", [DIN, DOUT], BF16, kind="ExternalInput").ap()
    atp = nc.dram_tensor("atp", [DIN, 128], BF16, kind="ExternalInput").ap()
    btp = nc.dram_tensor("btp", [128, DOUT], BF16, kind="ExternalInput").ap()
    biasc = nc.dram_tensor("biasc", [128, MT], F32, kind="ExternalInput").ap()
    out = nc.dram_tensor("out", [T, MT, 128, CTOK], BF16,
                         kind="ExternalOutput").ap()

    with tile.TileContext(nc) as tc, ExitStack() as ctx:
        const = ctx.enter_context(tc.tile_pool(name="const", bufs=1))
        brep_sb = ctx.enter_context(tc.tile_pool(name="brep_sb", bufs=3))
        out_sb = ctx.enter_context(tc.tile_pool(name="out_sb", bufs=4))
        bp_ps = ctx.enter_context(tc.tile_pool(name="bp_ps", bufs=2, space="PSUM"))
        dp_ps = ctx.enter_context(tc.tile_pool(name="dp_ps", bufs=3, space="PSUM"))

        # A + x on the sync ring; B/bias + W on the scalar HWDGE ring. Two
        # rings interleave at SDMA packet granularity, so the input DMA
        # prologue approaches the HBM floor instead of serializing per-DMA
        # completion receipts on one FIFO.
        at_all = const.tile([128, KT * 128], BF16, tag="at")
        nc.sync.dma_start(at_all.rearrange("p (k r) -> p k r", k=KT),
                          atp.rearrange("(k p) r -> p k r", p=128))
        bt_s = const.tile([128, DOUT], BF16, tag="bt")
        nc.scalar.dma_start(bt_s[:], btp[:, :])
        bias_s = const.tile([128, MT], F32, tag="bias")
        nc.scalar.dma_start(bias_s[:], biasc[:, :])

        at_t = [at_all[:, bass.ts(k, 128)] for k in range(KT)]
        xt_t = []
        wt_t = []
        for k in range(KT):
            tx = const.tile([128, CTOK], BF16, tag=f"xt{k}", name=f"tx{k}")
            nc.sync.dma_start(tx[:], xt[bass.ts(k, 128), :])
            xt_t.append(tx)
            tw = const.tile([128, DOUT], BF16, tag=f"wt{k}", name=f"tw{k}")
            nc.scalar.dma_start(tw[:], wt[bass.ts(k, 128), :])
            wt_t.append(tw)

        lowT_s = const.tile([128, CTOK], BF16, tag="lowT")

        # Warm-up on a memset tile: gates on no DMA, so the PE busy window
        # (HAM un-throttle needs ~3.4us sustained) starts immediately.
        wz = const.tile([128, 128], BF16, tag="wz")
        nc.vector.memset(wz[:], 0.0)
        warm = dp_ps.tile([128, 1024], F32, tag="dp", name="warm")
        for _ in range(40):
            nc.tensor.matmul(warm[:, 0:128], wz[:], wz[:],
                             start=True, stop=True)

        # Phase A (k-streamed): as (x_k, w_k) land, accumulate the low-rank
        # projection for all chunks and base m=0 chunks 0/1 k-outer.
        lowps = [dp_ps.tile([128, 1024], F32, tag="dp", name=f"lowps{g}")
                 for g in range(2)]
        bpA = [bp_ps.tile([128, 512], F32, tag="bp", name=f"bpA{c}")
               for c in range(2)]
        for k in range(KT):
            for c in range(NCH):
                nc.tensor.matmul(
                    lowps[c // 2][:, bass.ts(c % 2, 512)],
                    at_t[k][:],
                    xt_t[k][:, bass.ts(c, 512)],
                    start=(k == 0), stop=(k == KT - 1),
                )
            for c in range(2):
                nc.tensor.matmul(
                    bpA[c][:],
                    wt_t[k][:, 0:128],
                    xt_t[k][:, bass.ts(c, 512)],
                    start=(k == 0), stop=(k == KT - 1),
                )
        nc.scalar.copy(lowT_s[:, 0:1024], lowps[0][:])
        nc.scalar.copy(lowT_s[:, 1024:2048], lowps[1][:])

        # Main loop over (m, c); base(i) is emitted one step ahead of
        # delta(i-1)/adds(i-1) so the PE never head-of-line blocks on PSUM
        # granules still being drained by VectorE.
        mc = [(m, c) for m in range(MT) for c in range(NCH)]
        bps = {0: bpA[0], 1: bpA[1]}
        breps = {}

        def emit_base(i):
            m, c = mc[i]
            if i >= 2:
                bp = bp_ps.tile([128, 512], F32, tag="bp", name=f"bp{m}_{c}")
                for k in range(KT):
                    nc.tensor.matmul(
                        bp[:],
                        wt_t[k][:, bass.ts(m, 128)],
                        xt_t[k][:, bass.ts(c, 512)],
                        start=(k == 0), stop=(k == KT - 1),
                    )
                bps[i] = bp
            # Evacuate base twice (duplicated halves) with bias folded in, so
            # the FD=1024 adds read it without a broadcast AP.
            br = brep_sb.tile([128, 1024], F32, tag="brep", name=f"br{m}_{c}")
            for h in range(2):
                nc.scalar.activation(
                    br[:, bass.ts(h, 512)], bps[i][:],
                    mybir.ActivationFunctionType.Identity,
                    bias=bias_s[:, m:m + 1],
                )
            breps[i] = br

        out_r = out.rearrange("t m p x -> p m t x")

        def emit_delta_add(i):
            m, c = mc[i]
            # Per-chunk staging tile [128, t(4) x 512] bf16: both TT writes
            # and the store read are contiguous, and stores drain per chunk
            # instead of bunching at each m boundary.
            om = out_sb.tile([128, T * 512], BF16, tag="om", name=f"om{m}_{c}")
            gA = dp_ps.tile([128, 1024], F32, tag="dp", name=f"gA{m}_{c}")
            gB = dp_ps.tile([128, 1024], F32, tag="dp", name=f"gB{m}_{c}")
            halves = [gA[:, 0:512], gA[:, 512:1024],
                      gB[:, 0:512], gB[:, 512:1024]]
            for t in range(T):
                nc.tensor.matmul(
                    halves[t],
                    bt_s[32 * t:32 * t + R, bass.ts(m, 128)],
                    lowT_s[32 * t:32 * t + R, bass.ts(c, 512)],
                    start=True, stop=True,
                    tile_position=(32 * t, 0),
                )
            nc.vector.tensor_add(om[:, 0:1024], breps[i][:], gA[:])
            nc.vector.tensor_add(om[:, 1024:2048], breps[i][:], gB[:])
            nc.sync.dma_start(out_r[:, m, :, bass.ts(c, 512)],
                              om.rearrange("p (t x) -> p t x", t=T))

        for i in range(len(mc) + 1):
            if i < len(mc):
                emit_base(i)
            if i >= 1:
                emit_delta_add(i - 1)

    nc.compile()
    return nc


_NC = None


def _get_program():
    global _NC
    if _NC is None:
        _NC = _build_program()
    return _NC


def kernel(**inputs):
    x = np.ascontiguousarray(np.asarray(inputs["x"], dtype=np.float32))
    W = np.asarray(inputs["W"], dtype=np.float32)
    bias_v = np.asarray(inputs["bias"], dtype=np.float32)
    lora_A = np.asarray(inputs["lora_A"], dtype=np.float32)
    lora_B = np.asarray(inputs["lora_B"], dtype=np.float32)
    tuner_index = np.asarray(inputs["tuner_index"]).astype(np.int64)

    assert x.shape == (B, S, DIN) and W.shape == (DOUT, DIN)
    assert tuner_index.shape == (T,)

    A_sel = lora_A[tuner_index]          # [T, R, Din]
    B_sel = lora_B[tuner_index]          # [T, Dout, R]

    xT = np.ascontiguousarray(x.reshape(NTOK, DIN).T).astype(NPBF16)
    wt = np.ascontiguousarray(W.T).astype(NPBF16)       # [Din, Dout]
    atp = np.zeros((DIN, 128), NPBF16)
    atp.reshape(DIN, T, 32)[:, :, :R] = A_sel.transpose(2, 0, 1).astype(NPBF16)
    btp = np.zeros((128, DOUT), NPBF16)
    btp.reshape(T, 32, DOUT)[:, :R, :] = B_sel.transpose(0, 2, 1).astype(NPBF16)
    biasc = np.ascontiguousarray(bias_v.reshape(MT, 128).T)   # [128, MT]

    in_maps = []
    for c in range(NCORES):
        in_maps.append({
            "xt": np.ascontiguousarray(xT[:, c * CTOK:(c + 1) * CTOK]),
            "wt": wt,
            "atp": atp,
            "btp": btp,
            "biasc": biasc,
        })

    nc = _get_program()
    res = bass_utils.run_bass_kernel_spmd(nc, in_maps, core_ids=list(range(NCORES)))

    big = np.empty((T, MT, 128, NTOK), np.float32)
    for c in range(NCORES):
        big[:, :, :, c * CTOK:(c + 1) * CTOK] = res.results[c]["out"]
    # [T, m, p, tok] -> [T, tok, m*128+p]
    full = np.ascontiguousarray(big.transpose(0, 3, 1, 2))
    return full.reshape(T, B, S, DOUT)


# revision 8
# speedup vs baseline: 1.3225x; 1.0765x over previous
"""LoRA-linear Trainium2 Bass kernel (bf16 I/O, k-streamed prologue).

Computes, for T adapters: out[t] = x @ W.T + (x @ A_t.T) @ B_t.T + bias
Output: [T, B, S, Dout] float32 (device stores bf16; host upcasts).

Sharding: data-parallel over tokens across 8 NeuronCores (2048 tokens/core);
W/bias/selected-LoRA replicated. Matmul inputs are bf16 (host-cast);
accumulation stays fp32; outputs stored bf16 (abs error ~half-ulp(4.5)
≈ 0.008 ≪ the 0.09 budget at rel<2e-2).

Per-core layout puts Dout on PSUM partitions (out.T tiles [dout=128, tok]):
  lowT[32t+j, tok] = sum_d A_t[j,d] x[tok,d]   (PE, k-streamed)
  base.T[m]  = W[m-tile] @ x.T                 (PE, 8 k-tile accumulation)
  delta.T[t,m] = B_t.T row-group matmuls (K=16, tile_position=(32t,0), the
               four adapters issue back-to-back into distinct PSUM banks so
               3 run concurrently per the XBUS budget)
  out.T[t,m] = base.T[m] + delta.T[t,m]        (VectorE tensor_add)

Schedule:
  - Small tensors (A) load first on the sync ring so warm-up matmuls are not
    queued behind the 6 MB of x/W traffic; B/bias ride the scalar ring.
  - Phase A streams k-tiles: as (x_k, w_k) land, the low-rank matmuls and the
    first two base chunks accumulate k-outer, so the DMA prologue is filled
    with real PE work instead of pure warm-up.
  - Main loop per (m, c-chunk): 8 base matmuls -> 2 ScalarE activations
    evacuate base (bias folded) into a duplicated [128,1024] tile -> 4 delta
    matmuls -> 2 VectorE adds of FD=1024 (batching the four FD=512 adds into
    two halves the per-op PSUM overhead and keeps DVE off the critical path).
  - Stores are [128, 2048] bf16 blocks per (t, m), issued as each m finishes.
"""

import sys

if "/opt/trn_rl_repo" not in sys.path:
    sys.path.insert(0, "/opt/trn_rl_repo")

from contextlib import ExitStack

import ml_dtypes
import numpy as np

import concourse.bacc as bacc
import concourse.bass as bass
import concourse.mybir as mybir
import concourse.tile as tile
from concourse import bass_utils

# Problem constants (hardcoded per spec).
B, S, DIN, DOUT, R, NL, T = 4, 4096, 1024, 1024, 16, 8, 4
NCORES = 8
NTOK = B * S                 # 16384
CTOK = NTOK // NCORES        # 2048 tokens per core
KT = DIN // 128              # 8 k-tiles
MT = DOUT // 128             # 8 dout-tiles
NCH = CTOK // 512            # 4 token-chunks of 512

F32 = mybir.dt.float32
BF16 = mybir.dt.bfloat16
NPBF16 = ml_dtypes.bfloat16


def _build_program():
    nc = bacc.Bacc("TRN2", target_bir_lowering=False, debug=False,
                   num_devices=NCORES)

    xt = nc.dram_tensor("xt", [DIN, CTOK], BF16, kind="ExternalInput").ap()
    wt = nc.dram_tensor("wt", [DIN, DOUT], BF16, kind="ExternalInput").ap()
    atp = nc.dram_tensor("atp", [DIN, 128], BF16, kind="ExternalInput").ap()
    btp = nc.dram_tensor("btp", [128, DOUT], BF16, kind="ExternalInput").ap()
    biasc = nc.dram_tensor("biasc", [128, MT], F32, kind="ExternalInput").ap()
    out = nc.dram_tensor("out", [T, MT, 128, CTOK], BF16,
                         kind="ExternalOutput").ap()

    with tile.TileContext(nc) as tc, ExitStack() as ctx:
        const = ctx.enter_context(tc.tile_pool(name="const", bufs=1))
        brep_sb = ctx.enter_context(tc.tile_pool(name="brep_sb", bufs=3))
        out_sb = ctx.enter_context(tc.tile_pool(name="out_sb", bufs=4))
        bp_ps = ctx.enter_context(tc.tile_pool(name="bp_ps", bufs=2, space="PSUM"))
        dp_ps = ctx.enter_context(tc.tile_pool(name="dp_ps", bufs=3, space="PSUM"))

        # A + x on the sync ring; B/bias + W on the scalar HWDGE ring. Two
        # rings interleave at SDMA packet granularity, so the input DMA
        # prologue approaches the HBM floor instead of serializing per-DMA
        # completion receipts on one FIFO.
        at_all = const.tile([128, KT * 128], BF16, tag="at")
        nc.scalar.dma_start(at_all.rearrange("p (k r) -> p k r", k=KT),
                            atp.rearrange("(k p) r -> p k r", p=128))
        bt_s = const.tile([128, DOUT], BF16, tag="bt")
        nc.scalar.dma_start(bt_s[:], btp[:, :])
        bias_s = const.tile([128, MT], F32, tag="bias")
        nc.scalar.dma_start(bias_s[:], biasc[:, :])

        at_t = [at_all[:, bass.ts(k, 128)] for k in range(KT)]
        xt_t = []
        wt_t = []
        for k in range(KT):
            tx = const.tile([128, CTOK], BF16, tag=f"xt{k}", name=f"tx{k}")
            nc.sync.dma_start(tx[:], xt[bass.ts(k, 128), :])
            xt_t.append(tx)
            tw = const.tile([128, DOUT], BF16, tag=f"wt{k}", name=f"tw{k}")
            nc.scalar.dma_start(tw[:], wt[bass.ts(k, 128), :])
            wt_t.append(tw)

        lowT_s = const.tile([128, CTOK], BF16, tag="lowT")

        # Warm-up on a memset tile: gates on no DMA, so the PE busy window
        # (HAM un-throttle needs ~3.4us sustained) starts immediately.
        wz = const.tile([128, 128], BF16, tag="wz")
        nc.vector.memset(wz[:], 0.0)
        warm = dp_ps.tile([128, 1024], F32, tag="dp", name="warm")
        for _ in range(40):
            nc.tensor.matmul(warm[:, 0:128], wz[:], wz[:],
                             start=True, stop=True)

        # Phase A (k-streamed): as (x_k, w_k) land, accumulate the low-rank
        # projection for all chunks and base m=0 chunks 0/1 k-outer.
        lowps = [dp_ps.tile([128, 1024], F32, tag="dp", name=f"lowps{g}")
                 for g in range(2)]
        bpA = [bp_ps.tile([128, 512], F32, tag="bp", name=f"bpA{c}")
               for c in range(2)]
        for k in range(KT):
            for c in range(NCH):
                nc.tensor.matmul(
                    lowps[c // 2][:, bass.ts(c % 2, 512)],
                    at_t[k][:],
                    xt_t[k][:, bass.ts(c, 512)],
                    start=(k == 0), stop=(k == KT - 1),
                )
            for c in range(2):
                nc.tensor.matmul(
                    bpA[c][:],
                    wt_t[k][:, 0:128],
                    xt_t[k][:, bass.ts(c, 512)],
                    start=(k == 0), stop=(k == KT - 1),
                )
        # Evacuate lowT chunk-by-chunk on the (idle) VectorE so ScalarE is
        # free for the first base evacuations and chunk 0's delta matmuls can
        # start as soon as its 512 columns land.
        for c in range(NCH):
            nc.vector.tensor_copy(lowT_s[:, bass.ts(c, 512)],
                                  lowps[c // 2][:, bass.ts(c % 2, 512)])

        # Main loop over (m, c); base(i) is emitted one step ahead of
        # delta(i-1)/adds(i-1) so the PE never head-of-line blocks on PSUM
        # granules still being drained by VectorE.
        mc = [(m, c) for m in range(MT) for c in range(NCH)]
        bps = {0: bpA[0], 1: bpA[1]}
        breps = {}

        def emit_base(i):
            m, c = mc[i]
            if i >= 2:
                bp = bp_ps.tile([128, 512], F32, tag="bp", name=f"bp{m}_{c}")
                for k in range(KT):
                    nc.tensor.matmul(
                        bp[:],
                        wt_t[k][:, bass.ts(m, 128)],
                        xt_t[k][:, bass.ts(c, 512)],
                        start=(k == 0), stop=(k == KT - 1),
                    )
                bps[i] = bp
            # Evacuate base twice (duplicated halves) with bias folded in, so
            # the FD=1024 adds read it without a broadcast AP.
            br = brep_sb.tile([128, 1024], F32, tag="brep", name=f"br{m}_{c}")
            for h in range(2):
                nc.scalar.activation(
                    br[:, bass.ts(h, 512)], bps[i][:],
                    mybir.ActivationFunctionType.Identity,
                    bias=bias_s[:, m:m + 1],
                )
            breps[i] = br

        out_r = out.rearrange("t m p x -> p m t x")

        def emit_delta_add(i):
            m, c = mc[i]
            # Per-chunk staging tile [128, t(4) x 512] bf16: both TT writes
            # and the store read are contiguous, and stores drain per chunk
            # instead of bunching at each m boundary.
            om = out_sb.tile([128, T * 512], BF16, tag="om", name=f"om{m}_{c}")
            gA = dp_ps.tile([128, 1024], F32, tag="dp", name=f"gA{m}_{c}")
            gB = dp_ps.tile([128, 1024], F32, tag="dp", name=f"gB{m}_{c}")
            halves = [gA[:, 0:512], gA[:, 512:1024],
                      gB[:, 0:512], gB[:, 512:1024]]
            for t in range(T):
                nc.tensor.matmul(
                    halves[t],
                    bt_s[32 * t:32 * t + R, bass.ts(m, 128)],
                    lowT_s[32 * t:32 * t + R, bass.ts(c, 512)],
                    start=True, stop=True,
                    tile_position=(32 * t, 0),
                )
            nc.vector.tensor_add(om[:, 0:1024], breps[i][:], gA[:])
            nc.vector.tensor_add(om[:, 1024:2048], breps[i][:], gB[:])
            nc.sync.dma_start(out_r[:, m, :, bass.ts(c, 512)],
                              om.rearrange("p (t x) -> p t x", t=T))

        for i in range(len(mc) + 1):
            if i < len(mc):
                emit_base(i)
            if i >= 1:
                emit_delta_add(i - 1)

    nc.compile()
    return nc


_NC = None


def _get_program():
    global _NC
    if _NC is None:
        _NC = _build_program()
    return _NC


def kernel(**inputs):
    x = np.ascontiguousarray(np.asarray(inputs["x"], dtype=np.float32))
    W = np.asarray(inputs["W"], dtype=np.float32)
    bias_v = np.asarray(inputs["bias"], dtype=np.float32)
    lora_A = np.asarray(inputs["lora_A"], dtype=np.float32)
    lora_B = np.asarray(inputs["lora_B"], dtype=np.float32)
    tuner_index = np.asarray(inputs["tuner_index"]).astype(np.int64)

    assert x.shape == (B, S, DIN) and W.shape == (DOUT, DIN)
    assert tuner_index.shape == (T,)

    A_sel = lora_A[tuner_index]          # [T, R, Din]
    B_sel = lora_B[tuner_index]          # [T, Dout, R]

    xT = np.ascontiguousarray(x.reshape(NTOK, DIN).T).astype(NPBF16)
    wt = np.ascontiguousarray(W.T).astype(NPBF16)       # [Din, Dout]
    atp = np.zeros((DIN, 128), NPBF16)
    atp.reshape(DIN, T, 32)[:, :, :R] = A_sel.transpose(2, 0, 1).astype(NPBF16)
    btp = np.zeros((128, DOUT), NPBF16)
    btp.reshape(T, 32, DOUT)[:, :R, :] = B_sel.transpose(0, 2, 1).astype(NPBF16)
    biasc = np.ascontiguousarray(bias_v.reshape(MT, 128).T)   # [128, MT]

    in_maps = []
    for c in range(NCORES):
        in_maps.append({
            "xt": np.ascontiguousarray(xT[:, c * CTOK:(c + 1) * CTOK]),
            "wt": wt,
            "atp": atp,
            "btp": btp,
            "biasc": biasc,
        })

    nc = _get_program()
    res = bass_utils.run_bass_kernel_spmd(nc, in_maps, core_ids=list(range(NCORES)))

    big = np.empty((T, MT, 128, NTOK), np.float32)
    for c in range(NCORES):
        big[:, :, :, c * CTOK:(c + 1) * CTOK] = res.results[c]["out"]
    # [T, m, p, tok] -> [T, tok, m*128+p]
    full = np.ascontiguousarray(big.transpose(0, 3, 1, 2))
    return full.reshape(T, B, S, DOUT)
